# revision 42
# baseline (speedup 1.0000x reference)
"""Trainium2 Bass kernel for nn_PoincareConcatLinear (v2, fp16 + fused DVE).

Math (reference reformulated, bias==0, rc=sqrt(c)):
  Front-end collapses to x2 = x * Phi'_{t,s} with per-(token,stack) scalars
  (Phi' = Phi * 2*s1 folds the MLR row scale sigma_t = 2/(1-cx2) in, so the
  GEMM output IS u):
    u   = x2 @ wz            (wz = rc * weight_v / ||col||, fp16 matmul)
    L   = ln(|u| + sqrt(1+u^2)) = arsinh(|u|)
    w   = (2 g_j) * L        (g2b tile holds 2*weight_g)
    ds  = sign(u) * sinh(w)  (sinh via w*(1+w^2/6), |w|<~0.16 so err<3e-6)
    Q_t = sum_j ds^2
    out = ds * min(1/(rc*(1+sqrt(1+Q))), (1-eps)/(rc*sqrt(Q)))
  Derivation: y2*rc = sinh(2 g arsinh(u)) = ds; c*y2^2 = ds^2; the project()
  clip folds into the alpha min().

Engine split per row tile [128 tok, 2048 out]:
  PE : 32 matmuls fp16 (N=512)            ACT: Copy(mm)->um fp16 (frees PSUM),
  DVE: u2=um*um (2x), Sa=|um|+r1 (custom), Ln(1+u2), Exp(.5 lnq), Ln(Sa)
       w=L*g2b (2x), ds=SINH_SIGN (custom),
       q accum (stt), alpha scale (4x)
All transcendentals stay in the one natural_log_exp ACT table set.
"""
import math

import numpy as np

N_CORES = 8
N_TOK = 16384
TOK_PC = N_TOK // N_CORES      # 2048 tokens per core
R_TILES = TOK_PC // 128        # 16 row tiles
IN_STACKS, IN_DIM = 4, 256
K = IN_STACKS * IN_DIM         # 1024
KT = K // 128                  # 8
OUT_DIM = 2048
HALF = 1024
NH = OUT_DIM // HALF           # 2

EPS_PROJ = 1.0 - 0.004         # 0.996
USE_POOL_W = False             # GPSIMD steals the DVE SBUF port - net loss


def _beta(a, b):
    return math.exp(math.lgamma(a) + math.lgamma(b) - math.lgamma(a + b))


BETA_RATIO = _beta(K / 2.0, 0.5) / _beta(IN_DIM / 2.0, 0.5)

_CACHE = {}


def _pin_act_table_set():
    """Restrict walrus to the one ACT table set covering ln/exp/square, so it
    never ping-pongs ACT_TABLE_LOADs between sets (~2.7us each)."""
    import json
    import os
    import shutil
    import tempfile

    if os.environ.get("BASS_ACT_ROOT_JSON_PATH"):
        return
    try:
        import neuronxcc
        src = os.path.join(os.path.dirname(neuronxcc.__file__),
                           "pwp", "pwp_bin_trainium")
        info = json.load(open(os.path.join(src, "act_info.json")))
        keep = [e for e in info["act_func_sets"]
                if e["name"] == "natural_log_exp_and_others"]
        if not keep:
            return
        dst = tempfile.mkdtemp(prefix="act_single_")
        for e in keep:
            for k in info["pwp_file_keys"]:
                shutil.copy(os.path.join(src, e[k]), os.path.join(dst, e[k]))
        json.dump({"pwp_file_keys": info["pwp_file_keys"],
                   "act_func_sets": keep},
                  open(os.path.join(dst, "act_info.json"), "w"))
        os.environ["BASS_ACT_ROOT_JSON_PATH"] = os.path.join(dst, "act_info.json")
        import concourse.hw_specs as hw_specs
        import concourse.bacc as bacc_mod
        import concourse.mybir as mybir
        single = {
            e["name"]: {mybir.ActivationFunctionType.from_pwp(v)
                        for v in e["act"].keys()}
            for e in keep
        }
        hw_specs.get_activation_tables = lambda arch: single
        bacc_mod.get_activation_tables = lambda arch: single
    except Exception:
        pass


_DVE_OPS = {}


def _register_custom_dve():
    """Register fused DVE ops:
      ABS_ADD_ANT:   out = |Src0| + Src1
      SINH_SIGN_ANT: p = Src0*(1 + Src0^2*C0); out = sign(Src1)*p
    """
    if _DVE_OPS:
        return
    from concourse import dve_ops
    from concourse.dve_spec import Spec, Src0, Src1, C0, Zero, One, select, sq

    def _mk(name, spec):
        op = dve_ops.DveOp(name, spec, subdim=False, uops_sha={})
        dve_ops.OPS.append(op)
        dve_ops.CUSTOM_DVE_SPECS[name] = op.spec
        dve_ops._SUB_OPCODE_FOR_NAME[name] = (
            dve_ops._CUSTOM_DVE_ROW_BASE + len(dve_ops.OPS) - 1)
        for ver in ("v3", "v4"):
            try:
                op.compile(ver)
            except ValueError as e:
                import re
                m = re.search(r"\(%s: ([0-9a-f]+)" % ver, str(e))
                if m:
                    op.uops_sha[ver] = m.group(1)
                    op.compile(ver)
        return op

    def mk(name, body):
        return _mk(name, Spec(body=body))

    def mk_accum(name, body, accum):
        return _mk(name, Spec(body=body, accum=accum))

    _DVE_OPS["abs_add"] = mk(
        "ABS_ADD_ANT", select(Src0 >= Zero, Src0 + Src1, Src1 - Src0))
    p = Src0 * (One + sq(Src0) * C0)
    _DVE_OPS["sinh_sign"] = mk(
        "SINH_SIGN_ANT", select(Src1 >= Zero, p, Zero - p))
    from concourse.dve_spec import AluOp
    _DVE_OPS["qsq"] = mk_accum("QSQ_ANT", sq(Src0), AluOp.ADD)


def _build(c_val: float):
    import concourse.bacc as bacc
    import concourse.mybir as mybir
    import concourse.tile as tile

    _pin_act_table_set()
    _register_custom_dve()

    AF = mybir.ActivationFunctionType
    OP = mybir.AluOpType
    F32 = mybir.dt.float32
    F16 = mybir.dt.float16

    rc = math.sqrt(c_val)
    beta = BETA_RATIO

    nc = bacc.Bacc("TRN2", target_bir_lowering=False, debug=False,
                   num_devices=N_CORES)
    # xt/wz come host-packed to the SBUF layout [128, k*cols] so each loads
    # with one DMA of 32KB-contiguous per-partition descriptors
    xs = nc.declare_dram_parameter("xs", [TOK_PC, K], F16, isOutput=False)
    xt = nc.declare_dram_parameter("xt", [128, KT * TOK_PC], F16,
                                   isOutput=False)
    wz = nc.declare_dram_parameter("wz", [128, KT * OUT_DIM], F16,
                                   isOutput=False)
    gam = nc.declare_dram_parameter("gam", [1, OUT_DIM], F16, isOutput=False)
    out = nc.declare_dram_parameter("out", [TOK_PC, OUT_DIM], F16,
                                    isOutput=True)

    with tile.TileContext(nc) as tc:
        with (
            tc.tile_pool(name="const", bufs=1) as cpool,
            tc.tile_pool(name="wpool", bufs=1) as wpool,
            tc.tile_pool(name="xin", bufs=2) as xin,
            tc.tile_pool(name="x2r", bufs=2) as x2rp,
            tc.tile_pool(name="phib", bufs=1) as phib,
            tc.tile_pool(name="tiny", bufs=1) as tiny,
            tc.tile_pool(name="post", bufs=2) as post,
            tc.tile_pool(name="dpool", bufs=6) as dpool,
            tc.tile_pool(name="tailp", bufs=2) as tailp,
            tc.tile_pool(name="psmm", bufs=3, space="PSUM") as psmm,
        ):
            phis = nc.dram_tensor("phis", [IN_STACKS, TOK_PC], F16)

            # weights + x^T resident; their (big) DMAs are issued after
            # front_batch(0)'s loads so the front chain starts immediately
            wzr = wpool.tile([128, KT * OUT_DIM], F16, name="wzr")
            wzr3 = wzr[:].rearrange("p (k n) -> p k n", k=KT)
            xtp = wpool.tile([128, KT * TOK_PC], F16, name="xtp")
            xtp3 = xtp[:].rearrange("p (k t) -> p k t", k=KT)

            g2b = cpool.tile([128, OUT_DIM], F16, name="g2b")
            nc.sync.dma_start(out=g2b[:],
                              in_=gam[0:1, :].partition_broadcast(128))

            # ---------------- front-end (batched by 8 row-tiles) -----------
            RB = 8                      # row-tiles per batch
            NB = R_TILES // RB          # 2 batches
            BT = RB * 128               # tokens per batch (1024)
            W16 = RB * IN_STACKS        # 32

            def act(o, i, f, **kw):
                nc.scalar.activation(o, i, f, **kw)

            scl2 = tiny.tile([128, R_TILES], F32, name="scl2")
            qrow = tiny.tile([128, R_TILES], F32, name="qrow")
            alpha = tiny.tile([128, R_TILES], F32, name="alpha")

            phib_tiles = {}

            def front_batch(b):
                """Returns a list of 7 closures; running all of them issues
                the full front-end for batch b (x2r lands in x2r_byb[b]).
                Split so the chunks interleave with post work in the
                per-engine queues instead of forming one blocking run."""
                rsl = slice(b * RB, (b + 1) * RB)

                def tnew(nm, w=W16):
                    return tiny.tile([128, w], F32, tag=f"tb_{nm}", bufs=2,
                                     name=f"{nm}_b{b}")
                ssq = tnew("ssq")
                ssq3 = ssq[:].rearrange("p (r s) -> p r s", s=IN_STACKS)
                st = {}

                def mk_ssq(j):
                    def ssq_chunk():
                        for rb in (2 * j, 2 * j + 1):
                            r = b * RB + rb
                            xsb = xin.tile([128, K], F16, tag="xsb",
                                           name=f"xsb{r}")
                            nc.sync.dma_start(
                                out=xsb[:],
                                in_=xs[r * 128:(r + 1) * 128, :])
                            sqt = xin.tile([128, K], F16, tag="sqt",
                                           name=f"sqt{r}")
                            nc.vector.tensor_tensor(out=sqt[:], in0=xsb[:],
                                                    in1=xsb[:], op=OP.mult)
                            nc.vector.tensor_reduce(
                                out=ssq3[:, rb],
                                in_=sqt[:].rearrange("p (s d) -> p s d",
                                                     d=IN_DIM),
                                axis=mybir.AxisListType.X, op=OP.add)
                    return ssq_chunk

                def chunk1():
                    lnssq = tnew("lnssq")
                    act(lnssq[:], ssq[:], AF.Ln, scale=c_val)
                    un = tnew("un")
                    act(un[:], lnssq[:], AF.Exp, scale=0.5)
                    e2 = tnew("e2")
                    act(e2[:], un[:], AF.Exp, scale=-2.0)
                    onem = tnew("onem")
                    nc.vector.tensor_scalar(out=onem[:], in0=e2[:],
                                            scalar1=-1.0, scalar2=1.0,
                                            op0=OP.mult, op1=OP.add)
                    onep = tnew("onep")
                    nc.vector.tensor_scalar(out=onep[:], in0=e2[:],
                                            scalar1=1.0, scalar2=None,
                                            op0=OP.add)
                    rp = tnew("rp")
                    nc.vector.reciprocal(rp[:], onep[:])
                    tt_ = tnew("tt_")
                    nc.vector.tensor_tensor(out=tt_[:], in0=onem[:],
                                            in1=rp[:], op=OP.mult)
                    tc_ = tnew("tc_")
                    nc.vector.tensor_scalar(out=tc_[:], in0=tt_[:],
                                            scalar1=EPS_PROJ, scalar2=None,
                                            op0=OP.min)
                    l1 = tnew("l1")
                    act(l1[:], tc_[:], AF.Ln, scale=1.0, bias=1.0)
                    l2 = tnew("l2")
                    act(l2[:], tc_[:], AF.Ln, scale=-1.0, bias=1.0)
                    at2 = tnew("at2")
                    nc.vector.tensor_tensor(out=at2[:], in0=l1[:], in1=l2[:],
                                            op=OP.subtract)
                    run_ = tnew("run_")
                    nc.vector.reciprocal(run_[:], un[:])
                    ph1 = tnew("ph1")
                    nc.vector.tensor_tensor(out=ph1[:], in0=at2[:],
                                            in1=run_[:], op=OP.mult)
                    at2sq = tnew("at2sq")
                    nc.vector.tensor_tensor(out=at2sq[:], in0=at2[:],
                                            in1=at2[:], op=OP.mult)
                    st["ph1"] = ph1
                    st["at2sq"] = at2sq

                def chunk2():
                    ph1, at2sq = st["ph1"], st["at2sq"]
                    s4 = tnew("s4", RB)
                    nc.vector.tensor_reduce(
                        out=s4[:],
                        in_=at2sq[:].rearrange("p (r s) -> p r s",
                                               s=IN_STACKS),
                        axis=mybir.AxisListType.X, op=OP.add)
                    ls4 = tnew("ls4", RB)
                    act(ls4[:], s4[:], AF.Ln, scale=beta * beta / 4.0)
                    rcwn = tnew("rcwn", RB)
                    act(rcwn[:], ls4[:], AF.Exp, scale=0.5)
                    e2b = tnew("e2b", RB)
                    act(e2b[:], rcwn[:], AF.Exp, scale=-2.0)
                    onem2 = tnew("onem2", RB)
                    nc.vector.tensor_scalar(out=onem2[:], in0=e2b[:],
                                            scalar1=-1.0, scalar2=1.0,
                                            op0=OP.mult, op1=OP.add)
                    onep2 = tnew("onep2", RB)
                    nc.vector.tensor_scalar(out=onep2[:], in0=e2b[:],
                                            scalar1=1.0, scalar2=None,
                                            op0=OP.add)
                    rp2 = tnew("rp2", RB)
                    nc.vector.reciprocal(rp2[:], onep2[:])
                    t2_ = tnew("t2_", RB)
                    nc.vector.tensor_tensor(out=t2_[:], in0=onem2[:],
                                            in1=rp2[:], op=OP.mult)
                    t2c = tnew("t2c", RB)
                    nc.vector.tensor_scalar(out=t2c[:], in0=t2_[:],
                                            scalar1=EPS_PROJ, scalar2=None,
                                            op0=OP.min)
                    rrc = tnew("rrc", RB)
                    nc.vector.reciprocal(rrc[:], rcwn[:])
                    fac = tnew("fac", RB)
                    nc.vector.scalar_tensor_tensor(
                        out=fac[:], in0=t2c[:], scalar=beta / 2.0,
                        in1=rrc[:], op0=OP.mult, op1=OP.mult)
                    phi = tnew("phi")
                    phi3 = phi[:].rearrange("p (r s) -> p r s", s=IN_STACKS)
                    at23 = ph1[:].rearrange("p (r s) -> p r s", s=IN_STACKS)
                    for s in range(IN_STACKS):
                        nc.vector.tensor_tensor(out=phi3[:, :, s],
                                                in0=at23[:, :, s],
                                                in1=fac[:], op=OP.mult)
                    d2 = tnew("d2", RB)
                    nc.vector.tensor_tensor(out=d2[:], in0=t2c[:],
                                            in1=t2c[:], op=OP.mult)
                    omc = tnew("omc", RB)
                    nc.vector.tensor_scalar(out=omc[:], in0=d2[:],
                                            scalar1=-1.0, scalar2=1.0,
                                            op0=OP.mult, op1=OP.add)
                    omcc = tnew("omcc", RB)
                    nc.vector.tensor_scalar(out=omcc[:], in0=omc[:],
                                            scalar1=1e-15, scalar2=None,
                                            op0=OP.max)
                    s1v = tnew("s1v", RB)
                    nc.vector.reciprocal(s1v[:], omcc[:])
                    nc.vector.tensor_scalar(out=scl2[:, rsl], in0=s1v[:],
                                            scalar1=2.0, scalar2=None,
                                            op0=OP.mult)
                    # phi16 = phi * sigma_row, fp16 (folds the MLR row scale
                    # into the GEMM input), scattered to DRAM row-major
                    phi16 = tiny.tile([128, W16], F16, tag="phi16", bufs=3,
                                      name=f"phi16_{b}")
                    ph163 = phi16[:].rearrange("p (r s) -> p r s",
                                               s=IN_STACKS)
                    for rb in range(RB):
                        r = b * RB + rb
                        nc.vector.tensor_scalar(
                            out=ph163[:, rb], in0=phi3[:, rb],
                            scalar1=scl2[:, r:r + 1], scalar2=None,
                            op0=OP.mult)
                    for rb in range(RB):
                        nc.sync.dma_start(
                            out=phis[:, b * BT + rb * 128:
                                     b * BT + (rb + 1) * 128].rearrange(
                                         "s t -> t s"),
                            in_=phi16[:, rb * IN_STACKS:(rb + 1) * IN_STACKS])

                def chunk3():
                    for s in range(IN_STACKS):
                        pb = phib.tile([128, BT], F16, tag=f"ps{s}",
                                       name=f"phib{s}_{b}")
                        nc.sync.dma_start(
                            out=pb[:],
                            in_=phis[s:s + 1, b * BT:(b + 1) * BT
                                     ].partition_broadcast(128))
                        phib_tiles[(s, b)] = pb
                    x2r = x2rp.tile([128, KT * BT], F16, tag="x2r",
                                    name=f"x2r{b}")
                    x2r3 = x2r[:].rearrange("p (k t) -> p k t", k=KT)
                    for kk in range(KT):
                        nc.vector.tensor_tensor(
                            out=x2r3[:, kk],
                            in0=xtp3[:, kk, b * BT:(b + 1) * BT],
                            in1=phib_tiles[(kk // 2, b)][:], op=OP.mult)
                    x2r_byb[b] = x2r3

                return [mk_ssq(0), mk_ssq(1), mk_ssq(2), mk_ssq(3),
                        chunk1, chunk2, chunk3]

            # ---------------- main loop: one row tile = [128, 2048] --------
            GROUP = 4

            x2r_byb = {}
            fb0 = front_batch(0)
            for fc in fb0[:-1]:
                fc()
            # resident loads go to the DMA queues behind the first xsb loads
            # but ahead of chunk3's x2r reads of xtp
            nc.sync.dma_start(out=wzr[:], in_=wz[:, :])
            nc.sync.dma_start(out=xtp[:], in_=xt[:, :])
            fb0[-1]()
            um_t, u2_t, lnq_t, r1_t, sa_t, l_t, w_t = ({} for _ in range(7))
            ds_t = {}

            def stage_a(r):
                """GEMM both halves + PSUM->SBUF fp16 copy (keeps sign)."""
                b, rb = r // RB, r % RB
                x2r3 = x2r_byb[b]
                um = post.tile([128, OUT_DIM], F16, tag="um", name=f"um{r}")
                um_t[r] = um
                for h in range(NH):
                    mm = psmm.tile([128, HALF], F32, tag="mm",
                                   name=f"mm{r}_{h}")
                    for nb_ in range(HALF // 512):
                        for kk in range(KT):
                            nc.tensor.matmul(
                                mm[:, nb_ * 512:(nb_ + 1) * 512],
                                x2r3[:, kk, rb * 128:(rb + 1) * 128],
                                wzr3[:, kk, h * HALF + nb_ * 512:
                                     h * HALF + (nb_ + 1) * 512],
                                start=(kk == 0), stop=(kk == KT - 1))
                    act(um[:, h * HALF:(h + 1) * HALF], mm[:, :], AF.Copy)

            def pnew(nm, r):
                return post.tile([128, OUT_DIM], F16, tag=f"po_{nm}",
                                 name=f"{nm}{r}")

            def stage_b1(r):
                """u2 (DVE), lnq+r1 (ACT)."""
                um = um_t[r]
                u2 = pnew("u2", r)
                nc.vector.tensor_tensor(out=u2[:], in0=um[:], in1=um[:],
                                        op=OP.mult)
                u2_t[r] = u2
                lnq = pnew("lnq", r)
                act(lnq[:], u2[:], AF.Ln, scale=1.0, bias=1.0)
                lnq_t[r] = lnq
                r1 = pnew("r1", r)
                act(r1[:], lnq[:], AF.Exp, scale=0.5)
                r1_t[r] = r1

            def stage_b2(r):
                """Sa (DVE custom), L (ACT)."""
                sa = pnew("sa", r)
                nc.vector._custom_dve(
                    _DVE_OPS["abs_add"], out=sa[:], in0=um_t[r][:],
                    in1=r1_t[r][:])
                sa_t[r] = sa
                L = pnew("L", r)
                act(L[:], sa[:], AF.Ln)
                l_t[r] = L

            def stage_c(r):
                """w, ds, q accum (DVE; the g2b mult rides on GPSIMD)."""
                w = pnew("w", r)
                eng = nc.gpsimd if USE_POOL_W else nc.vector
                eng.tensor_tensor(out=w[:], in0=l_t[r][:], in1=g2b[:],
                                  op=OP.mult)
                ds = dpool.tile([128, OUT_DIM], F16, tag="ds", name=f"ds{r}")
                nc.vector._custom_dve(
                    _DVE_OPS["sinh_sign"], out=ds[:], in0=w[:],
                    in1=um_t[r][:], s0=1.0 / 6.0)
                ds_t[r] = ds
                # qsq's streamed output is garbage; write it over w (dead)
                nc.vector._custom_dve(
                    _DVE_OPS["qsq"], out=w[:], in0=ds[:],
                    accum_out=qrow[:, r:r + 1])

            def tail(g0, g1):
                """alpha for rows [g0, g1), then scale + DMA out."""
                GW = g1 - g0
                qs = qrow[:, g0:g1]

                def gnew(name):
                    return tailp.tile([128, GW], F32, tag=f"tail_{name}",
                                      name=f"{name}_{g0}")
                qg = gnew("qg")
                nc.vector.tensor_scalar(out=qg[:], in0=qs, scalar1=1e-30,
                                        scalar2=None, op0=OP.max)
                # alpha_d = 1/(rc*(1+sqrt(1+Q)))
                lb = gnew("lb")
                act(lb[:], qg[:], AF.Ln, scale=1.0, bias=1.0)
                sb_ = gnew("sb_")
                act(sb_[:], lb[:], AF.Exp, scale=0.5)
                sb2 = gnew("sb2")
                nc.vector.tensor_scalar(out=sb2[:], in0=sb_[:], scalar1=1.0,
                                        scalar2=None, op0=OP.add)
                rsb = gnew("rsb")
                nc.vector.reciprocal(rsb[:], sb2[:])
                ad = gnew("ad")
                nc.vector.tensor_scalar(out=ad[:], in0=rsb[:],
                                        scalar1=1.0 / rc, scalar2=None,
                                        op0=OP.mult)
                # alpha_c = (0.996/rc)/sqrt(Q)
                lq = gnew("lq")
                act(lq[:], qg[:], AF.Ln)
                rq = gnew("rq")
                act(rq[:], lq[:], AF.Exp, scale=-0.5)
                ac = gnew("ac")
                nc.vector.tensor_scalar(out=ac[:], in0=rq[:],
                                        scalar1=EPS_PROJ / rc, scalar2=None,
                                        op0=OP.mult)
                nc.vector.tensor_tensor(out=alpha[:, g0:g1], in0=ad[:],
                                        in1=ac[:], op=OP.min)
                for rr in range(g0, g1):
                    nc.vector.tensor_scalar(
                        out=ds_t[rr][:], in0=ds_t[rr][:],
                        scalar1=alpha[:, rr:rr + 1], scalar2=None,
                        op0=OP.mult)
                    nc.sync.dma_start(
                        out=out[rr * 128:(rr + 1) * 128, :],
                        in_=ds_t[rr][:])
                    del ds_t[rr]

            # software pipeline: keep both engines' queues from head-of-line
            # blocking on each other (stage_c(r-1) sits between u2/Sa ACT
            # round trips of row r); front chunks for batch b+2 interleave
            # one per row so they never form a blocking run
            stage_a(0)
            pend = front_batch(1)
            for r in range(R_TILES):
                if r < len(pend):
                    pend[r]()
                if r + 1 < R_TILES:
                    stage_a(r + 1)
                stage_b1(r)
                if r >= 1:
                    stage_c(r - 1)
                    if r % GROUP == 0:
                        tail(r - GROUP, r)
                stage_b2(r)
            stage_c(R_TILES - 1)
            tail(R_TILES - GROUP, R_TILES)

    nc.compile()
    return nc


def _pack_kmajor(a):
    """[K, N] -> [128, KT*N] so partition p's row is contiguous in DRAM:
    out[p, kk*N + n] = a[kk*128 + p, n]."""
    n = a.shape[1]
    return np.ascontiguousarray(
        a.reshape(KT, 128, n).transpose(1, 0, 2).reshape(128, KT * n))


def _prep_inputs(x, weight_g, weight_v, c_val):
    rc = math.sqrt(c_val)
    norms = np.maximum(np.linalg.norm(weight_v, axis=0), 1e-15)
    wz = _pack_kmajor((rc * weight_v / norms[None, :]).astype(np.float16))
    gam = np.ascontiguousarray(
        (2.0 * weight_g)[None, :].astype(np.float16))
    xf = x.reshape(N_TOK, K)
    in_maps = []
    for cix in range(N_CORES):
        shard = xf[cix * TOK_PC:(cix + 1) * TOK_PC]
        in_maps.append({
            "xs": np.ascontiguousarray(shard.astype(np.float16)),
            "xt": _pack_kmajor(
                np.ascontiguousarray(shard.T.astype(np.float16))),
            "wz": wz,
            "gam": gam,
        })
    return in_maps


def _numpy_fallback(x, weight_g, weight_v, bias, c):
    """Pure-numpy mirror of the reference (used only if bias != 0)."""
    x = x.astype(np.float64)
    c = float(c)
    rc = math.sqrt(c)

    def project(v, k, eps=0.004):
        maxnorm = (1.0 - eps) / math.sqrt(abs(k))
        n = np.maximum(np.linalg.norm(v, axis=-1, keepdims=True), 1e-15)
        return np.where(n > maxnorm, v / n * maxnorm, v)

    def expmap0(u):
        un = np.maximum(np.linalg.norm(u, axis=-1, keepdims=True), 1e-15)
        return project(np.tanh(rc * un) * u / (rc * un), -c)

    def logmap0(y):
        yn = np.maximum(np.linalg.norm(y, axis=-1, keepdims=True), 1e-15)
        arg = np.minimum(rc * yn, 1.0 - 1e-7)
        return y / (rc * yn) * np.arctanh(arg)

    xb = expmap0(x)
    xb = logmap0(xb).reshape(x.shape[0], K)
    xb = expmap0(xb * BETA_RATIO)
    norms = np.maximum(np.linalg.norm(weight_v, axis=0), 1e-15)
    zu = weight_v / norms
    rcx = rc * xb
    cx2 = np.sum(rcx * rcx, axis=-1, keepdims=True)
    drcr = 2.0 * rc * bias.astype(np.float64)
    num = (2.0 * (rcx @ zu) * np.cosh(drcr)
           - (1.0 + cx2) * np.sinh(drcr))
    arg = num / np.maximum(1.0 - cx2, 1e-15)
    y = 2.0 * weight_g / rc * np.arcsinh(arg)
    y2 = np.sinh(rc * y) / rc
    denom = 1.0 + np.sqrt(1.0 + c * np.sum(y2 * y2, axis=-1, keepdims=True))
    return project(y2 / denom, -c).astype(np.float32)


def kernel(x, weight_g, weight_v, bias, c):
    from concourse.bass_utils import run_bass_kernel_spmd

    x = np.ascontiguousarray(np.asarray(x, dtype=np.float32))
    weight_g = np.asarray(weight_g, dtype=np.float32)
    weight_v = np.asarray(weight_v, dtype=np.float32)
    bias = np.asarray(bias, dtype=np.float32)
    c_val = float(np.asarray(c, dtype=np.float32))
    if not bool(np.all(bias == 0.0)):
        return _numpy_fallback(x, weight_g, weight_v, bias, c_val)

    if c_val not in _CACHE:
        _CACHE[c_val] = _build(c_val)
    nc = _CACHE[c_val]

    in_maps = _prep_inputs(x, weight_g, weight_v, c_val)
    res = run_bass_kernel_spmd(nc, in_maps, list(range(N_CORES)))
    outs = [res.results[cix]["out"].astype(np.float32)
            for cix in range(N_CORES)]
    return np.concatenate(outs, axis=0)


def profile(inputs, trace_kwargs=None):
    """Run once with NTFF tracing, return hw exec time in ns (core 0)."""
    from concourse.bass_utils import run_bass_kernel_spmd

    _install_profile_hook()
    x = np.asarray(inputs["x"], dtype=np.float32)
    weight_g = np.asarray(inputs["weight_g"], dtype=np.float32)
    weight_v = np.asarray(inputs["weight_v"], dtype=np.float32)
    c_val = float(np.asarray(inputs["c"], dtype=np.float32))
    if c_val not in _CACHE:
        _CACHE[c_val] = _build(c_val)
    nc = _CACHE[c_val]
    in_maps = _prep_inputs(x, weight_g, weight_v, c_val)
    res = run_bass_kernel_spmd(nc, in_maps, list(range(N_CORES)), trace=True,
                               **(trace_kwargs or {}))
    return res.exec_time_ns


def _install_profile_hook():
    """Make antenv.axon_hooks importable + registered (profiling only)."""
    import sys
    import types
    try:
        from antenv.axon_hooks import get_axon_ntff_profile_hook
        if get_axon_ntff_profile_hook() is not None:
            return
    except ImportError:
        m = types.ModuleType("antenv.axon_hooks")
        m._hook = None
        m.set_axon_ntff_profile_hook = lambda h: setattr(m, "_hook", h)
        m.get_axon_ntff_profile_hook = lambda: m._hook
        sys.modules["antenv.axon_hooks"] = m
    try:
        from trn_agent_boot.trn_boot import _ntff_profile_via_ctypes
        from antenv.axon_hooks import (get_axon_ntff_profile_hook,
                                       set_axon_ntff_profile_hook)
        if get_axon_ntff_profile_hook() is None:
            set_axon_ntff_profile_hook(
                _ntff_profile_via_ctypes("/opt/axon/libaxon_pjrt.so"))
    except Exception:
        pass
